# revision 5
# baseline (speedup 1.0000x reference)
"""Single-head attention (B=4, S=4096, D=1024, N=L=128) on 8 trn2 NeuronCores.

Sharding: core c handles batch b = c//2, query half h = c%2 (2048 queries).
Each core receives the full context of its batch with its own query half
ordered FIRST (attention is permutation-invariant over context), computes
k/v projections over all 4096 context tokens and q over its 2048 queries,
then scores^T = k^T.T @ q^T per 512-query block, exp on ACT (1/sqrt(D)
folded into the activation scale), PV accumulation + ones-row column-sum
matmuls, normalization, PE transpose back to [q, l], DMA out.

Matmuls run as float32r (full-rate PE path, ~1e-4 component relative error);
set MM_DT = mybir.dt.float32 for the 4x-slower full-precision variant.
"""
from contextlib import ExitStack

import numpy as np

import concourse.bass as bass  # noqa: F401  (bass types via bacc)
import concourse.tile as tile
import concourse.mybir as mybir
from concourse import bacc
from concourse.bass_utils import run_bass_kernel_spmd
from concourse.masks import make_identity

B, S, D, N, L = 4, 4096, 1024, 128, 128
NCORES = 8
SQ = B * S // NCORES      # 2048 queries per core
CCH = 512                 # context chunk (tokens) in phase 1
QB = 512                  # query block in phase 2
NCH = S // CCH            # 8 context chunks
NKC = S // 128            # 32 key/value chunks of 128
ND = D // 128             # 8 contraction tiles over D
SCALE = 1.0 / float(np.sqrt(D))

MM_DT = mybir.dt.float32r
F32 = mybir.dt.float32


def emit(nc, tc, ctx, x, wqt, wkt, wvt, out):
    persist = ctx.enter_context(tc.tile_pool(name="persist", bufs=1))
    ident32 = persist.tile([128, 128], F32, tag="ident32")
    make_identity(nc, ident32)
    if MM_DT != F32:
        ident = persist.tile([128, 128], MM_DT, tag="ident")
        nc.vector.tensor_copy(ident, ident32)
    else:
        ident = ident32
    ones32 = persist.tile([128, 1], F32, tag="ones32")
    nc.vector.memset(ones32, 1.0)
    if MM_DT != F32:
        ones = persist.tile([128, 1], MM_DT, tag="ones")
        nc.vector.tensor_copy(ones, ones32)
    else:
        ones = ones32

    kT = persist.tile([128, S], MM_DT, tag="kT")        # [n, kctx]
    vv = persist.tile([128, S], MM_DT, tag="vv")        # 32 chunks [kctx128, l]
    qT = persist.tile([128, SQ], MM_DT, tag="qT")       # [n, q]

    w_tiles = {}
    for nm, w in (("q", wqt), ("k", wkt), ("v", wvt)):
        for d in range(ND):
            t = persist.tile([128, N], MM_DT, tag=f"w{nm}{d}")
            nc.sync.dma_start(out=t, in_=w[d * 128:(d + 1) * 128, :])
            w_tiles[nm, d] = t

    # ---------------- phase 1: x^T + projections ----------------
    with (
        tc.tile_pool(name="p1", bufs=2) as p1,
        tc.tile_pool(name="p1x", bufs=3) as p1x,
        tc.tile_pool(name="p1ps", bufs=2, space="PSUM") as p1ps,
        tc.tile_pool(name="p1px", bufs=2, space="PSUM") as p1px,
    ):
        for c in range(NCH):
            tok0 = c * CCH
            xts = []
            for t in range(CCH // 128):
                xt = p1x.tile([128, D], MM_DT, tag=f"x{t}")
                nc.sync.dma_start(
                    out=xt, in_=x[tok0 + t * 128: tok0 + (t + 1) * 128, :])
                xts.append(xt)
            xT = []
            for d in range(ND):
                px = p1px.tile([128, CCH], MM_DT, tag="px")
                for t in range(CCH // 128):
                    nc.tensor.transpose(
                        px[:, t * 128:(t + 1) * 128],
                        xts[t][:, d * 128:(d + 1) * 128], ident)
                xTd = p1.tile([128, CCH], MM_DT, tag=f"xT{d}")
                nc.vector.tensor_copy(xTd, px)
                xT.append(xTd)

            csl = slice(tok0, tok0 + CCH)
            pk = p1ps.tile([128, CCH], F32, tag="pk")
            for d in range(ND):
                nc.tensor.matmul(pk, w_tiles["k", d][:], xT[d][:],
                                 start=(d == 0), stop=(d == ND - 1))
            nc.vector.tensor_copy(kT[:, csl], pk)

            pv = p1ps.tile([128, CCH], F32, tag="pv")
            for d in range(ND):
                nc.tensor.matmul(pv, w_tiles["v", d][:], xT[d][:],
                                 start=(d == 0), stop=(d == ND - 1))
            vTc = p1.tile([128, CCH], MM_DT, tag="vTc")
            nc.vector.tensor_copy(vTc, pv)
            pvt = p1px.tile([128, CCH], MM_DT, tag="px")
            for t in range(CCH // 128):
                nc.tensor.transpose(
                    pvt[:, t * 128:(t + 1) * 128],
                    vTc[:, t * 128:(t + 1) * 128], ident)
            nc.vector.tensor_copy(vv[:, csl], pvt)

            if c < NCH // 2:
                pq = p1ps.tile([128, CCH], F32, tag="pq")
                for d in range(ND):
                    nc.tensor.matmul(pq, w_tiles["q", d][:], xT[d][:],
                                     start=(d == 0), stop=(d == ND - 1))
                nc.vector.tensor_copy(qT[:, csl], pq)

    # ---------------- phase 2: attention ----------------
    with (
        tc.tile_pool(name="p2s", bufs=1) as p2s,
        tc.tile_pool(name="p2w", bufs=2) as p2w,
        tc.tile_pool(name="ps_s", bufs=2, space="PSUM") as ps_s_pool,
        tc.tile_pool(name="ps_o", bufs=2, space="PSUM") as ps_o_pool,
        tc.tile_pool(name="ps_r", bufs=1, space="PSUM") as ps_r_pool,
        tc.tile_pool(name="ps_t", bufs=2, space="PSUM") as ps_t_pool,
    ):
        for b in range(SQ // QB):
            qsl = slice(b * QB, (b + 1) * QB)
            sTs = []
            for i in range(NKC):
                pss = ps_s_pool.tile([128, QB], F32, tag="pss")
                nc.tensor.matmul(pss, kT[:, i * 128:(i + 1) * 128],
                                 qT[:, qsl], start=True, stop=True)
                sT = p2s.tile([128, QB], MM_DT, tag=f"sT{i}")
                nc.scalar.activation(sT, pss,
                                     func=mybir.ActivationFunctionType.Exp,
                                     scale=SCALE)
                sTs.append(sT)

            po = ps_o_pool.tile([128, QB], F32, tag="po")     # outT accum
            prs = ps_r_pool.tile([1, QB], F32, tag="prs")     # colsum accum
            for i in range(NKC):
                nc.tensor.matmul(po, vv[:, i * 128:(i + 1) * 128], sTs[i][:],
                                 start=(i == 0), stop=(i == NKC - 1))
                nc.tensor.matmul(prs, ones[:], sTs[i][:],
                                 start=(i == 0), stop=(i == NKC - 1))

            # 1/colsum, transposed to [q, 1] layout
            ssb = p2w.tile([1, QB], F32, tag="ssb")
            nc.vector.tensor_copy(ssb, prs)
            sumT = p2w.tile([128, QB // 128], F32, tag="sumT")
            for j in range(QB // 128):
                pt = ps_t_pool.tile([128, 128], F32, tag="pt")
                nc.tensor.transpose(pt[:, 0:1],
                                    ssb[0:1, j * 128:(j + 1) * 128],
                                    ident32[0:1, 0:1])
                nc.vector.tensor_copy(sumT[:, j:j + 1], pt[:, 0:1])
            recipT = p2w.tile([128, QB // 128], F32, tag="recipT")
            nc.vector.reciprocal(recipT, sumT)

            for j in range(QB // 128):
                ot = p2w.tile([128, 128], F32, tag="ot")
                nc.vector.tensor_copy(ot, po[:, j * 128:(j + 1) * 128])
                ptr = ps_t_pool.tile([128, 128], F32, tag="pt")
                nc.tensor.transpose(ptr, ot, ident32)
                fo = p2w.tile([128, 128], F32, tag="fo")
                nc.vector.tensor_scalar_mul(fo, in0=ptr[:, :],
                                            scalar1=recipT[:, j:j + 1])
                nc.sync.dma_start(
                    out=out[b * QB + j * 128: b * QB + (j + 1) * 128, :],
                    in_=fo)


def build_bass(iters=1):
    nc = bacc.Bacc()
    x = nc.dram_tensor("x_part", [S, D], MM_DT, kind="ExternalInput")
    wqt = nc.dram_tensor("wqt", [D, N], MM_DT, kind="ExternalInput")
    wkt = nc.dram_tensor("wkt", [D, N], MM_DT, kind="ExternalInput")
    wvt = nc.dram_tensor("wvt", [D, L], MM_DT, kind="ExternalInput")
    out = nc.dram_tensor("out_part", [SQ, L], F32, kind="ExternalOutput")
    with tile.TileContext(nc) as tc:
        for _ in range(iters):
            with ExitStack() as ctx:
                emit(nc, tc, ctx, x, wqt, wkt, wvt, out)
    nc.compile()
    return nc


def make_in_maps(x, Wq, Wk, Wv):
    wqt = np.ascontiguousarray(Wq.T)
    wkt = np.ascontiguousarray(Wk.T)
    wvt = np.ascontiguousarray(Wv.T)
    in_maps = []
    for c in range(NCORES):
        bb, h = c // 2, c % 2
        xb = x[bb]
        x_part = xb if h == 0 else np.ascontiguousarray(
            np.concatenate([xb[SQ:], xb[:SQ]], axis=0))
        in_maps.append({"x_part": np.ascontiguousarray(x_part),
                        "wqt": wqt, "wkt": wkt, "wvt": wvt})
    return in_maps


def kernel(x, Wq, Wk, Wv):
    x = np.asarray(x, dtype=np.float32)
    nc = build_bass()
    res = run_bass_kernel_spmd(
        nc, make_in_maps(x, np.asarray(Wq, np.float32),
                         np.asarray(Wk, np.float32),
                         np.asarray(Wv, np.float32)),
        core_ids=list(range(NCORES)))
    out = np.empty((B, S, L), dtype=np.float32)
    for c in range(NCORES):
        bb, h = c // 2, c % 2
        out[bb, h * SQ:(h + 1) * SQ] = res.results[c]["out_part"]
    return out


# revision 19
# speedup vs baseline: 1821.0252x; 1821.0252x over previous
"""Single-head attention (B=4, S=4096, D=1024, N=L=128) on 8 trn2 NeuronCores.

Sharding: core c handles batch b = c//2, query half h = c%2 (2048 queries).
Each core receives the full context of its batch with its own query half
ordered FIRST (attention is permutation-invariant over the context axis), so
one SPMD program serves all cores with no dynamic indexing: k/v projections
over all 4096 context tokens, q over rows 0:2048.

Per-core pipeline:
  phase 1: PE-transpose x into x^T tiles (contraction over D needs D on the
           partition axis), then k^T = WkT.T @ x^T, v^T -> v chunks via PE
           transpose, q^T — all accumulated in PSUM over 8 D-tiles.
  phase 2: per 512-query block, scores^T = kT_chunk.T @ qT (one PSUM bank
           per kctx chunk, two banks exp'd per ACT instruction to amortize
           access latency; 1/sqrt(D) folded into the activation scale),
           PV accumulation po += v_chunk.T @ sT on PE, softmax denominators
           via wide partial adds split across DVE/GPSIMD plus one small
           fp32 ones-matmul, then normalize, PE-transpose back to [q, l],
           DMA out.

Matmuls run as float32r (full-rate PE path; end-to-end max relative error
vs the fp32 reference measured at 2.2e-4). Set MM_DT = mybir.dt.float32 for
the full-precision variant (2.2e-6, ~2.3x slower).
"""
from contextlib import ExitStack

import numpy as np

import concourse.tile as tile
import concourse.mybir as mybir
from concourse import bacc
from concourse.bass_utils import run_bass_kernel_spmd
from concourse.masks import make_identity

B, S, D, N, L = 4, 4096, 1024, 128, 128
NCORES = 8
SQ = B * S // NCORES      # 2048 queries per core
CCH = 512                 # context chunk (tokens) in phase 1
QB = 512                  # query block in phase 2
NCH = S // CCH            # 8 context chunks
NKC = S // 128            # 32 key/value chunks of 128
ND = D // 128             # 8 contraction tiles over D
GRP = 2                   # kctx chunks exp'd per ACT instruction
SCALE = 1.0 / float(np.sqrt(D))

MM_DT = mybir.dt.float32r
F32 = mybir.dt.float32


def emit(nc, tc, ctx, x, wqt, wkt, wvt, out):
    persist = ctx.enter_context(tc.tile_pool(name="persist", bufs=1))
    ident32 = persist.tile([128, 128], F32, tag="ident32")
    make_identity(nc, ident32)
    ones32 = persist.tile([128, 1], F32, tag="ones32")
    nc.vector.memset(ones32, 1.0)
    if MM_DT != F32:
        # fp32r tiles must be produced by an instruction with float32r
        # output dtype (memset/ALU ops on fp32r tiles fail codegen)
        ident = persist.tile([128, 128], MM_DT, tag="ident")
        nc.vector.tensor_copy(ident, ident32)
    else:
        ident = ident32

    kT = persist.tile([128, S], MM_DT, tag="kT")       # [n, kctx]
    vv = persist.tile([128, S], MM_DT, tag="vv")       # 32 chunks [kctx128, l]
    qT = persist.tile([128, SQ], MM_DT, tag="qT")      # [n, q]

    w_tiles = {}
    for nm, w in (("q", wqt), ("k", wkt), ("v", wvt)):
        for d in range(ND):
            t = persist.tile([128, N], MM_DT, tag=f"w{nm}{d}")
            nc.sync.dma_start(out=t, in_=w[d * 128:(d + 1) * 128, :])
            w_tiles[nm, d] = t

    # ---------------- phase 1: x^T + projections ----------------
    with (
        tc.tile_pool(name="p1", bufs=2) as p1,
        tc.tile_pool(name="p1x", bufs=3) as p1x,
        tc.tile_pool(name="p1ps", bufs=2, space="PSUM") as p1ps,
        tc.tile_pool(name="p1px", bufs=4, space="PSUM") as p1px,
    ):
        for c in range(NCH):
            tok0 = c * CCH
            xts = []
            for t in range(CCH // 128):
                xt = p1x.tile([128, D], MM_DT, tag=f"x{t}")
                nc.sync.dma_start(
                    out=xt, in_=x[tok0 + t * 128: tok0 + (t + 1) * 128, :])
                xts.append(xt)
            xT = []
            for d in range(ND):
                px = p1px.tile([128, CCH], MM_DT, tag="px")
                for t in range(CCH // 128):
                    nc.tensor.transpose(
                        px[:, t * 128:(t + 1) * 128],
                        xts[t][:, d * 128:(d + 1) * 128], ident)
                xTd = p1.tile([128, CCH], MM_DT, tag=f"xT{d}")
                nc.vector.tensor_copy(xTd, px)
                xT.append(xTd)

            csl = slice(tok0, tok0 + CCH)
            pk = p1ps.tile([128, CCH], F32, tag="proj")
            for d in range(ND):
                nc.tensor.matmul(pk, w_tiles["k", d][:], xT[d][:],
                                 start=(d == 0), stop=(d == ND - 1))
            nc.vector.tensor_copy(kT[:, csl], pk)

            pv = p1ps.tile([128, CCH], F32, tag="proj")
            for d in range(ND):
                nc.tensor.matmul(pv, w_tiles["v", d][:], xT[d][:],
                                 start=(d == 0), stop=(d == ND - 1))
            vTc = p1.tile([128, CCH], MM_DT, tag="vTc")
            nc.vector.tensor_copy(vTc, pv)
            pvt = p1px.tile([128, CCH], MM_DT, tag="px")
            for t in range(CCH // 128):
                nc.tensor.transpose(
                    pvt[:, t * 128:(t + 1) * 128],
                    vTc[:, t * 128:(t + 1) * 128], ident)
            nc.vector.tensor_copy(vv[:, csl], pvt)

            if c < NCH // 2:
                pq = p1ps.tile([128, CCH], F32, tag="proj")
                for d in range(ND):
                    nc.tensor.matmul(pq, w_tiles["q", d][:], xT[d][:],
                                     start=(d == 0), stop=(d == ND - 1))
                nc.vector.tensor_copy(qT[:, csl], pq)

    # ---------------- phase 2: attention ----------------
    with (
        tc.tile_pool(name="p2s", bufs=1) as p2s,
        tc.tile_pool(name="p2w", bufs=2) as p2w,
        tc.tile_pool(name="ps_s", bufs=2, space="PSUM") as ps_s_pool,
        tc.tile_pool(name="ps_o", bufs=2, space="PSUM") as ps_o_pool,
        tc.tile_pool(name="ps_r", bufs=1, space="PSUM") as ps_r_pool,
        tc.tile_pool(name="ps_t", bufs=1, space="PSUM") as ps_t_pool,
    ):
        NG = NKC // GRP
        for b in range(SQ // QB):
            qsl = slice(b * QB, (b + 1) * QB)
            po = ps_o_pool.tile([128, QB], F32, tag="po")        # outT accum
            part0 = p2w.tile([128, GRP * QB], F32, tag="part0")  # colsum partials
            part1 = p2w.tile([128, GRP * QB], F32, tag="part1")
            sTg = []
            # group pipeline: scores+exp for group g overlap PV/sum for g-1
            for g in range(NG + 1):
                if g < NG:
                    pss = ps_s_pool.tile([128, GRP * QB], F32, tag="pss")
                    for u in range(GRP):
                        i = g * GRP + u
                        nc.tensor.matmul(pss[:, u * QB:(u + 1) * QB],
                                         kT[:, i * 128:(i + 1) * 128],
                                         qT[:, qsl], start=True, stop=True)
                    sT = p2s.tile([128, GRP * QB], MM_DT, tag=f"sT{g}")
                    nc.scalar.activation(sT, pss,
                                         func=mybir.ActivationFunctionType.Exp,
                                         scale=SCALE)
                    sTg.append(sT)
                if g >= 1:
                    h = g - 1
                    for u in range(GRP):
                        j = h * GRP + u
                        sl = sTg[h][:, u * QB:(u + 1) * QB]
                        nc.tensor.matmul(po, vv[:, j * 128:(j + 1) * 128],
                                         sl, start=(j == 0),
                                         stop=(j == NKC - 1),
                                         skip_group_check=True)
                    # per-partition partial colsums: wide adds split between
                    # DVE and GPSIMD (both otherwise idle-ish in phase 2)
                    part = part0 if (h % 2 == 0) else part1
                    eng = nc.vector if (h % 2 == 0) else nc.gpsimd
                    sl2 = sTg[h][:, :].bitcast(F32)
                    if h < 2:
                        eng.tensor_copy(part, sl2)
                    else:
                        eng.tensor_add(part, part, sl2)
            # combine chains, then fold the 128 partial rows into [1, QB]
            # with one small fp32 matmul against a ones vector
            prs = ps_r_pool.tile([1, QB], F32, tag="prs")
            nc.vector.tensor_add(part0, part0, part1)
            ps3 = part0.rearrange("p (u q) -> p u q", u=GRP)
            nc.tensor.matmul(prs, ones32[:], ps3[:, 0, :], start=True,
                             stop=False)
            nc.tensor.matmul(prs, ones32[:], ps3[:, 1, :], start=False,
                             stop=True)

            # 1/colsum, transposed to [q, 1] layout
            ssb = p2w.tile([1, QB], F32, tag="ssb")
            nc.vector.tensor_copy(ssb, prs)
            sumT = p2w.tile([128, QB // 128], F32, tag="sumT")
            for j in range(QB // 128):
                pt = ps_t_pool.tile([128, 128], F32, tag="pt")
                nc.tensor.transpose(pt[:, 0:1],
                                    ssb[0:1, j * 128:(j + 1) * 128],
                                    ident32[0:1, 0:1])
                nc.vector.tensor_copy(sumT[:, j:j + 1], pt[:, 0:1])
            recipT = p2w.tile([128, QB // 128], F32, tag="recipT")
            nc.vector.reciprocal(recipT, sumT)

            for j in range(QB // 128):
                ot = p2w.tile([128, 128], F32, tag="ot")
                nc.scalar.copy(out=ot[:, :], in_=po[:, j * 128:(j + 1) * 128])
                ptr = ps_t_pool.tile([128, 128], F32, tag="pt")
                nc.tensor.transpose(ptr, ot, ident32)
                fo = p2w.tile([128, 128], F32, tag="fo")
                nc.vector.tensor_scalar_mul(fo, in0=ptr[:, :],
                                            scalar1=recipT[:, j:j + 1])
                nc.sync.dma_start(
                    out=out[b * QB + j * 128: b * QB + (j + 1) * 128, :],
                    in_=fo)


def build_bass(iters=1):
    nc = bacc.Bacc()
    x = nc.dram_tensor("x_part", [S, D], MM_DT, kind="ExternalInput")
    wqt = nc.dram_tensor("wqt", [D, N], MM_DT, kind="ExternalInput")
    wkt = nc.dram_tensor("wkt", [D, N], MM_DT, kind="ExternalInput")
    wvt = nc.dram_tensor("wvt", [D, L], MM_DT, kind="ExternalInput")
    out = nc.dram_tensor("out_part", [SQ, L], F32, kind="ExternalOutput")
    with tile.TileContext(nc) as tc:
        for _ in range(iters):
            with ExitStack() as ctx:
                emit(nc, tc, ctx, x, wqt, wkt, wvt, out)
    nc.compile()
    return nc


def make_in_maps(x, Wq, Wk, Wv):
    wqt = np.ascontiguousarray(np.asarray(Wq, np.float32).T)
    wkt = np.ascontiguousarray(np.asarray(Wk, np.float32).T)
    wvt = np.ascontiguousarray(np.asarray(Wv, np.float32).T)
    x = np.asarray(x, np.float32)
    in_maps = []
    for c in range(NCORES):
        bb, h = c // 2, c % 2
        xb = x[bb]
        x_part = xb if h == 0 else np.concatenate([xb[SQ:], xb[:SQ]], axis=0)
        in_maps.append({"x_part": np.ascontiguousarray(x_part),
                        "wqt": wqt, "wkt": wkt, "wvt": wvt})
    return in_maps


def kernel(x, Wq, Wk, Wv):
    nc = build_bass()
    res = run_bass_kernel_spmd(nc, make_in_maps(x, Wq, Wk, Wv),
                               core_ids=list(range(NCORES)))
    out = np.empty((B, S, L), dtype=np.float32)
    for c in range(NCORES):
        bb, h = c // 2, c % 2
        out[bb, h * SQ:(h + 1) * SQ] = res.results[c]["out_part"]
    return out
